# revision 10
# baseline (speedup 1.0000x reference)
"""RetinaFace-style multi-task loss on 8 Trainium2 NeuronCores via Bass/Tile.

Data-parallel: 16 samples sharded 2-per-core across 8 cores. Two device
kernels per call:

  Phase A (Bass, dense): per sample, IoU of 102400 anchors x 32 GT boxes,
    running max + argmax, pos/neg thresholds, hard-negative-mined
    classification neg_mean via 5-round 16-way threshold search.
    Anchor tensor is embedded in the NEFF as a const (loaded to HBM once
    at model load; never re-shipped). Exports a packed uint8 plane
    (argmax | pos<<7) + per-sample stats.
  Host: decode packed plane, compact positive-anchor indices, slice the
    needed rows of bbox/ldm regressions (so the 1.25 GB ldm tensor never
    crosses the host-device link).
  Phase B (Bass, sparse): <=256 positive rows per sample. GT boxes and
    landmarks gathered from annotations on-device via one-hot matmul on
    the PE. SmoothL1 bbox loss + wing landmark loss + final combine.

Anchor layout on device: anchor a = p*800 + n  (p = SBUF partition,
n = free-dim column), so [102400, X] DRAM tensors reshape to [128, 800, X]
with fully contiguous per-partition DMA rows.
"""
import sys
import types
import numpy as np

_B, _A, _N = 16, 102400, 32
_NC = 8
_SPC = 2          # samples per core
_P, _NA = 128, 800  # A = P * NA
_K = 256          # max positives per sample handled on the sparse path
_OMEGA, _EPS = 3.0, 2.0
_WING_C = _OMEGA - _OMEGA * float(np.log(1.0 + _OMEGA / _EPS))
_VBIG = -100.0    # "minus infinity" for non-negative-anchor scores
_HI0 = 64.0       # upper bound for threshold search
_NITER = 5        # threshold-search rounds

_state = {}


# --------------------------------------------------------------------------
# Bass module builders
# --------------------------------------------------------------------------

def _build_phase_a(anchors_np):
    import concourse.tile as tile
    import concourse.mybir as mybir
    from concourse import bacc
    from concourse import bass_isa

    AL = mybir.AluOpType
    ACT = mybir.ActivationFunctionType
    f32 = mybir.dt.float32
    f16 = mybir.dt.float16
    u8 = mybir.dt.uint8
    i32 = mybir.dt.int32

    nc = bacc.Bacc("TRN2", target_bir_lowering=False, debug=False,
                   num_devices=_NC, name="loss_phase_a")
    cls1h = nc.dram_tensor("cls1h", [_SPC, _P, _NA], f16, kind="ExternalInput")
    binfo = nc.dram_tensor("binfo", [1, _SPC * _N * 8], f32, kind="ExternalInput")
    packed = nc.dram_tensor("packed", [_SPC, _P, _NA], u8, kind="ExternalOutput")
    stats = nc.dram_tensor("stats", [1, 8], f32, kind="ExternalOutput")

    anc_c = nc.inline_tensor(
        np.ascontiguousarray(anchors_np.reshape(_P, _NA * 4)), name="anc_c")
    coef_c = nc.inline_tensor(
        ((np.arange(16, dtype=np.float32) + 1.0) / 17.0).reshape(1, 16),
        name="coef_c")

    S = _SPC
    T24 = 24  # per-partition top-K kept for hard-negative mining
    with tile.TileContext(nc) as tc:
        with tc.tile_pool(name="keep", bufs=1) as cp, \
             tc.tile_pool(name="tmp", bufs=3) as tp, \
             tc.tile_pool(name="scr", bufs=4) as sp:
            def jt(nm, bufs=16):
                return tp.tile([_P, _NA], f32, tag="jt", name=nm, bufs=bufs)

            # ---- anchors -> contiguous planes
            anc_t = cp.tile([_P, _NA * 4], f32)
            nc.sync.dma_start(anc_t[:], anc_c[:])
            ancv = anc_t[:].rearrange("p (n c) -> p c n", c=4)
            ax = [cp.tile([_P, _NA], f32, tag=f"ax{q}", name=f"ax{q}")
                  for q in range(4)]
            for q in range(4):
                nc.vector.tensor_copy(ax[q][:], ancv[:, q, :])
            aw_t = jt("aw_t")
            nc.vector.tensor_tensor(aw_t[:], ax[2][:], ax[0][:], AL.subtract)
            ah_t = jt("ah_t")
            nc.vector.tensor_tensor(ah_t[:], ax[3][:], ax[1][:], AL.subtract)
            aarea = cp.tile([_P, _NA], f32)
            nc.vector.tensor_tensor(aarea[:], aw_t[:], ah_t[:], AL.mult)

            # ---- cls1 (fp16) -> negated fp32
            cls1_t = cp.tile([_P, S, _NA], f16)
            for s in range(S):
                nc.sync.dma_start(cls1_t[:, s, :], cls1h[s])
            vminus = cp.tile([_P, S, _NA], f32)
            nc.vector.tensor_scalar(vminus[:], cls1_t[:], -1.0, None, AL.mult)

            # ---- per-(s, j) box scalars, broadcast to all partitions
            b0 = cp.tile([1, S * _N * 8], f32)
            nc.sync.dma_start(b0[:], binfo[:])
            bb = cp.tile([_P, S * _N * 8], f32)
            nc.gpsimd.partition_broadcast(bb[:], b0[:])
            bv = bb[:].rearrange("p (s j q) -> p s j q", s=S, j=_N)

            def bq(s, j, q):  # [P, 1] per-partition scalar AP
                return bv[:, s, j, q:q + 1]

            # ---- IoU loop: running max of inter/ua as exact num/den pair
            num = [cp.tile([_P, _NA], f32, name=f"num{s}") for s in range(S)]
            den = [cp.tile([_P, _NA], f32, name=f"den{s}") for s in range(S)]
            arg = [cp.tile([_P, _NA], f32, name=f"arg{s}") for s in range(S)]
            for s in range(S):
                nc.vector.memset(num[s][:], -1.0)
                nc.vector.memset(den[s][:], 1.0)
                nc.vector.memset(arg[s][:], 0.0)

            for j in range(_N):
                for s in range(S):
                    # min/max vs box scalars: POOL; overlap arith: DVE;
                    # area add + relu: ACT
                    t1 = jt("t1")
                    nc.gpsimd.tensor_scalar(t1[:], ax[2][:], bq(s, j, 2),
                                            None, AL.min)
                    t2 = jt("t2")
                    nc.gpsimd.tensor_scalar(t2[:], ax[0][:], bq(s, j, 0),
                                            None, AL.max)
                    t3 = jt("t3")
                    nc.gpsimd.tensor_scalar(t3[:], ax[3][:], bq(s, j, 3),
                                            None, AL.min)
                    t4 = jt("t4")
                    nc.gpsimd.tensor_scalar(t4[:], ax[1][:], bq(s, j, 1),
                                            None, AL.max)
                    e = nc.gpsimd if s == 1 else nc.vector
                    iw = jt("iw")
                    e.tensor_tensor(iw[:], t1[:], t2[:], AL.subtract)
                    ih = jt("ih")
                    e.tensor_tensor(ih[:], t3[:], t4[:], AL.subtract)
                    ihc = jt("ihc")
                    nc.scalar.activation(ihc[:], ih[:], ACT.Relu)
                    u0 = jt("u0")
                    nc.scalar.activation(u0[:], aarea[:], ACT.Identity,
                                         bias=bq(s, j, 4))
                    inter = jt("inter")
                    nc.vector.scalar_tensor_tensor(inter[:], iw[:], 0.0,
                                                   ihc[:], AL.max, AL.mult)
                    u1 = jt("u1")
                    e.tensor_tensor(u1[:], u0[:], inter[:], AL.subtract)
                    ua = jt("ua")
                    e.tensor_scalar(ua[:], u1[:], 1e-8, None, AL.max)
                    gn = jt("gn")
                    e.tensor_tensor(gn[:], inter[:], den[s][:], AL.mult)
                    gd = jt("gd")
                    e.tensor_tensor(gd[:], num[s][:], ua[:], AL.mult)
                    gtf = jt("gtf")
                    nc.vector.tensor_tensor(gtf[:], gn[:], gd[:], AL.is_gt)
                    gti = tp.tile([_P, _NA], i32, tag="gti", name="gti",
                                  bufs=4)
                    nc.vector.tensor_copy(gti[:], gtf[:])
                    nc.vector.copy_predicated(num[s][:], gti[:], inter[:])
                    nc.vector.copy_predicated(den[s][:], gti[:], ua[:])
                    nc.vector.scalar_tensor_tensor(arg[s][:], gtf[:],
                                                   float(j), arg[s][:],
                                                   AL.mult, AL.max)

            # ---- pos / neg masks + counts (exact: num >= thr * den)
            cnt = cp.tile([_P, 4], f32)
            pos = cp.tile([_P, S, _NA], f32)
            neg = cp.tile([_P, S, _NA], f32)
            for s in range(S):
                th = sp.tile([_P, _NA], f32, tag="th", bufs=2, name="th")
                nc.vector.tensor_scalar(th[:], den[s][:], 0.7, None, AL.mult)
                thn = sp.tile([_P, _NA], f32, tag="thn", bufs=2, name="thn")
                nc.vector.tensor_scalar(thn[:], den[s][:], 0.4, None, AL.mult)
                posr = sp.tile([_P, _NA], f32, tag="posr", bufs=2, name="posr")
                nc.vector.tensor_tensor(posr[:], num[s][:], th[:], AL.is_ge)
                nc.vector.tensor_copy(pos[:, s, :], posr[:])
                nc.vector.tensor_reduce(cnt[:, s:s + 1], posr[:],
                                        axis=mybir.AxisListType.X, op=AL.add)
                negr = sp.tile([_P, _NA], f32, tag="negr", bufs=2, name="negr")
                nc.vector.tensor_tensor(negr[:], num[s][:], thn[:], AL.is_lt)
                nc.vector.tensor_copy(neg[:, s, :], negr[:])
                nc.vector.tensor_reduce(cnt[:, 2 + s:3 + s], negr[:],
                                        axis=mybir.AxisListType.X, op=AL.add)
            cntr = cp.tile([_P, 4], f32)
            nc.gpsimd.partition_all_reduce(cntr[:], cnt[:], channels=_P,
                                           reduce_op=bass_isa.ReduceOp.add)
            keep = cp.tile([_P, S], f32)
            nc.vector.scalar_tensor_tensor(keep[:], cntr[:, 0:2], 3.0,
                                           cntr[:, 2:4], AL.mult, AL.min)

            # ---- v = neg ? -cls1 : VBIG, then per-partition top-24
            v = cp.tile([_P, S, _NA], f32)
            nc.vector.memset(v[:], _VBIG)
            neg_i = cp.tile([_P, S, _NA], i32)
            nc.vector.tensor_copy(neg_i[:], neg[:])
            nc.vector.copy_predicated(v[:], neg_i[:], vminus[:])
            tops = cp.tile([_P, S, T24], f32)
            for s in range(S):
                vv = v[:, s, :]
                for r in range(T24 // 8):
                    nc.vector.max(tops[:, s, r * 8:(r + 1) * 8], vv)
                    if r < T24 // 8 - 1:
                        vn = sp.tile([_P, _NA], f32, tag="vmr", bufs=2,
                                     name="vmr")
                        nc.vector.match_replace(
                            vn[:], tops[:, s, r * 8:(r + 1) * 8], vv, -200.0)
                        vv = vn[:]

            # ---- threshold search on the compacted top values
            co0 = cp.tile([1, 16], f32)
            nc.sync.dma_start(co0[:], coef_c[:])
            cb = cp.tile([_P, 16], f32)
            nc.gpsimd.partition_broadcast(cb[:], co0[:])
            cbv = cb[:].rearrange("p (o k) -> p o k", o=1).broadcast_to(
                [_P, S, 16])
            p100 = cp.tile([_P, S, 16], f32)
            nc.vector.memset(p100[:], 100.0)
            lo = cp.tile([_P, S], f32)
            nc.vector.memset(lo[:], _VBIG)
            hi = cp.tile([_P, S], f32)
            nc.vector.memset(hi[:], _HI0)

            def b2(t):
                return t[:].rearrange("p (s o) -> p s o", o=1).broadcast_to(
                    [_P, S, 16])

            keep_b = b2(keep)
            for it in range(_NITER):
                d = tp.tile([_P, S], f32, tag="d")
                nc.vector.tensor_tensor(d[:], hi[:], lo[:], AL.subtract)
                tk = tp.tile([_P, S, 16], f32, tag="tk")
                nc.vector.tensor_tensor(tk[:], cbv, b2(d), AL.mult)
                nc.vector.tensor_tensor(tk[:], tk[:], b2(lo), AL.add)
                cnts = tp.tile([_P, S * 16], f32, tag="cnts")
                for s in range(S):
                    for k in range(16):
                        scr = sp.tile([_P, T24], f32, tag="scr")
                        nc.vector.tensor_scalar(
                            scr[:], tops[:, s, :], tk[:, s, k:k + 1], None,
                            AL.is_ge, AL.add,
                            accum_out=cnts[:, s * 16 + k:s * 16 + k + 1])
                cntsr = tp.tile([_P, S * 16], f32, tag="cntsr")
                nc.gpsimd.partition_all_reduce(
                    cntsr[:], cnts[:], channels=_P,
                    reduce_op=bass_isa.ReduceOp.add)
                cnv = cntsr[:].rearrange("p (s k) -> p s k", s=S)
                big = tp.tile([_P, S, 16], i32, tag="big")
                nc.vector.tensor_tensor(big[:], cnv, keep_b, AL.is_ge)
                bt = tp.tile([_P, S, 16], f32, tag="bt")
                nc.vector.memset(bt[:], _VBIG)
                nc.vector.copy_predicated(bt[:], big[:], tk[:])
                rmx = tp.tile([_P, S], f32, tag="rmx")
                nc.vector.tensor_reduce(rmx[:], bt[:],
                                        axis=mybir.AxisListType.X, op=AL.max)
                nc.vector.tensor_tensor(lo[:], lo[:], rmx[:], AL.max)
                ht = tp.tile([_P, S, 16], f32, tag="ht")
                nc.vector.tensor_copy(ht[:], tk[:])
                nc.vector.copy_predicated(ht[:], big[:], p100[:])
                rmn = tp.tile([_P, S], f32, tag="rmn")
                nc.vector.tensor_reduce(rmn[:], ht[:],
                                        axis=mybir.AxisListType.X, op=AL.min)
                nc.vector.tensor_tensor(hi[:], hi[:], rmn[:], AL.min)

            # ---- exact sum/count above lo (on compacted tops)
            sc4 = cp.tile([_P, 4], f32)
            for s in range(S):
                sel = sp.tile([_P, T24], f32, tag="sel", bufs=2)
                nc.vector.scalar_tensor_tensor(sel[:], tops[:, s, :],
                                               lo[:, s:s + 1], tops[:, s, :],
                                               AL.is_ge, AL.mult)
                nc.vector.tensor_reduce(sc4[:, s:s + 1], sel[:],
                                        axis=mybir.AxisListType.X, op=AL.add)
                scc = sp.tile([_P, T24], f32, tag="scc", bufs=2)
                nc.vector.tensor_scalar(scc[:], tops[:, s, :], lo[:, s:s + 1],
                                        None, AL.is_ge, AL.add,
                                        accum_out=sc4[:, 2 + s:3 + s])
            sc4r = cp.tile([_P, 4], f32)
            nc.gpsimd.partition_all_reduce(sc4r[:], sc4[:], channels=_P,
                                           reduce_op=bass_isa.ReduceOp.add)
            # neg_mean = (s_lo - (c_lo - keep) * lo) / max(keep, 1)
            e1 = tp.tile([_P, S], f32, tag="e1")
            nc.vector.tensor_tensor(e1[:], sc4r[:, 2:4], keep[:], AL.subtract)
            nc.vector.tensor_tensor(e1[:], e1[:], lo[:], AL.mult)
            nc.vector.tensor_tensor(e1[:], sc4r[:, 0:2], e1[:], AL.subtract)
            kf = tp.tile([_P, S], f32, tag="kf")
            nc.vector.tensor_scalar(kf[:], keep[:], 1.0, None, AL.max)
            rk = tp.tile([_P, S], f32, tag="rk")
            nc.vector.reciprocal(rk[:], kf[:])
            nm = cp.tile([_P, S], f32)
            nc.vector.tensor_tensor(nm[:], e1[:], rk[:], AL.mult)

            # ---- packed output
            pku = cp.tile([_P, S, _NA], u8)
            for s in range(S):
                pk = jt("pk")
                nc.vector.scalar_tensor_tensor(pk[:], pos[:, s, :], 128.0,
                                               arg[s][:], AL.mult, AL.add)
                nc.vector.tensor_copy(pku[:, s, :], pk[:])
                nc.sync.dma_start(packed[s], pku[:, s, :])
            nc.sync.dma_start(stats[:, 0:2], nm[0:1, :])
            nc.sync.dma_start(stats[:, 2:4], cntr[0:1, 0:2])
            nc.sync.dma_start(stats[:, 4:6], cntr[0:1, 2:4])
    nc.compile()
    return nc


def _build_phase_b():
    import concourse.tile as tile
    import concourse.mybir as mybir
    from concourse import bacc
    from concourse import bass_isa

    AL = mybir.AluOpType
    ACT = mybir.ActivationFunctionType
    f32 = mybir.dt.float32
    f16 = mybir.dt.float16

    S, C, R = _SPC, _K // _P, 196
    SC = S * C
    nc = bacc.Bacc("TRN2", target_bir_lowering=False, debug=False,
                   num_devices=_NC, name="loss_phase_b")
    lrh = nc.dram_tensor("lrh", [_P, SC * R], f16, kind="ExternalInput")
    brt = nc.dram_tensor("brt", [_P, SC * 4], f32, kind="ExternalInput")
    abt = nc.dram_tensor("abt", [_P, SC * 4], f32, kind="ExternalInput")
    c0t = nc.dram_tensor("c0t", [_P, SC], f32, kind="ExternalInput")
    rvt = nc.dram_tensor("rvt", [_P, SC], f32, kind="ExternalInput")
    lpt = nc.dram_tensor("lpt", [_P, SC], f32, kind="ExternalInput")
    agt = nc.dram_tensor("agt", [1, SC * _P], f32, kind="ExternalInput")
    ann2 = nc.dram_tensor("ann2", [S, _N, 200], f32, kind="ExternalInput")
    svec = nc.dram_tensor("svec", [1, 16], f32, kind="ExternalInput")
    out = nc.dram_tensor("out", [1, 8], f32, kind="ExternalOutput")

    ev = np.zeros((1, 3 * R), np.float32)
    ev[0, 0:R][0::2] = 1.0                      # even mask
    ev[0, R:2 * R][1::2] = 1.0                  # odd mask
    ev[0, 2 * R:2 * R + 68] = 1.0               # s vector
    ev[0, 2 * R + 68:3 * R] = 3.0
    ev_c = nc.inline_tensor(ev, name="ev_c")

    with tile.TileContext(nc) as tc:
        with tc.tile_pool(name="keep", bufs=1) as cp, \
             tc.tile_pool(name="tmp", bufs=3) as tp, \
             tc.tile_pool(name="ps", bufs=2, space="PSUM") as pp:
            # ---- on-device gather of GT boxes + landmarks via one-hot matmul
            annt = cp.tile([_N, S * 200], f32)
            for s in range(S):
                nc.sync.dma_start(annt[:, s * 200:(s + 1) * 200], ann2[s])
            ag0 = cp.tile([1, SC * _P], f32)
            nc.sync.dma_start(ag0[:], agt[:])
            agb = cp.tile([_N, SC * _P], f32)
            nc.gpsimd.partition_broadcast(agb[:], ag0[:], channels=_N)
            iot = cp.tile([_N, 1], mybir.dt.int32)
            nc.gpsimd.iota(iot[:], pattern=[[0, 1]], base=0,
                           channel_multiplier=1)
            iof = cp.tile([_N, 1], f32)
            nc.vector.tensor_copy(iof[:], iot[:])
            oh = cp.tile([_N, SC * _P], f32)
            nc.vector.tensor_tensor(oh[:], agb[:],
                                    iof[:].broadcast_to([_N, SC * _P]),
                                    AL.is_equal)
            gt_t = cp.tile([_P, SC, 200], f32)
            for s in range(S):
                for c in range(C):
                    sc = s * C + c
                    gps = pp.tile([_P, 200], f32, tag="gps")
                    nc.tensor.matmul(gps[:], oh[:, sc * _P:(sc + 1) * _P],
                                     annt[:, s * 200:(s + 1) * 200],
                                     start=True, stop=True)
                    nc.vector.tensor_copy(gt_t[:, sc, :], gps[:])

            # ---- anchor-row planes
            ab_t = cp.tile([_P, SC * 4], f32)
            nc.sync.dma_start(ab_t[:], abt[:])
            abv = ab_t[:].rearrange("p (sc c) -> p c sc", c=4)
            aX = [tp.tile([_P, SC], f32, tag=f"aX{q}", name=f"aX{q}")
                  for q in range(4)]
            for q in range(4):
                nc.vector.tensor_copy(aX[q][:], abv[:, q, :])
            aw = cp.tile([_P, SC], f32)
            nc.vector.tensor_tensor(aw[:], aX[2][:], aX[0][:], AL.subtract)
            ah = cp.tile([_P, SC], f32)
            nc.vector.tensor_tensor(ah[:], aX[3][:], aX[1][:], AL.subtract)
            acx = cp.tile([_P, SC], f32)
            nc.vector.scalar_tensor_tensor(acx[:], aw[:], 0.5, aX[0][:],
                                           AL.mult, AL.add)
            acy = cp.tile([_P, SC], f32)
            nc.vector.scalar_tensor_tensor(acy[:], ah[:], 0.5, aX[1][:],
                                           AL.mult, AL.add)
            gx = gt_t[:].rearrange("p sc d -> p d sc")  # strided views
            gw = cp.tile([_P, SC], f32)
            nc.vector.tensor_tensor(gw[:], gx[:, 2, :], gx[:, 0, :],
                                    AL.subtract)
            gh = cp.tile([_P, SC], f32)
            nc.vector.tensor_tensor(gh[:], gx[:, 3, :], gx[:, 1, :],
                                    AL.subtract)
            gcx = tp.tile([_P, SC], f32, tag="gcx")
            nc.vector.scalar_tensor_tensor(gcx[:], gw[:], 0.5, gx[:, 0, :],
                                           AL.mult, AL.add)
            gcy = tp.tile([_P, SC], f32, tag="gcy")
            nc.vector.scalar_tensor_tensor(gcy[:], gh[:], 0.5, gx[:, 1, :],
                                           AL.mult, AL.add)

            # ---- bbox targets & SmoothL1
            btgt = cp.tile([_P, SC, 4], f32)
            awe = tp.tile([_P, SC], f32, tag="awe")
            nc.vector.tensor_scalar(awe[:], aw[:], 1e-14, None, AL.add)
            rwe = tp.tile([_P, SC], f32, tag="rwe")
            nc.vector.reciprocal(rwe[:], awe[:])
            ahe = tp.tile([_P, SC], f32, tag="ahe")
            nc.vector.tensor_scalar(ahe[:], ah[:], 1e-14, None, AL.add)
            rhe = tp.tile([_P, SC], f32, tag="rhe")
            nc.vector.reciprocal(rhe[:], ahe[:])
            tmp1 = tp.tile([_P, SC], f32, tag="tmp1")
            nc.vector.tensor_tensor(tmp1[:], gcx[:], acx[:], AL.subtract)
            nc.vector.tensor_tensor(tmp1[:], tmp1[:], rwe[:], AL.mult)
            nc.vector.tensor_scalar(btgt[:, :, 0], tmp1[:], 10.0, None,
                                    AL.mult)
            tmp2 = tp.tile([_P, SC], f32, tag="tmp2")
            nc.vector.tensor_tensor(tmp2[:], gcy[:], acy[:], AL.subtract)
            nc.vector.tensor_tensor(tmp2[:], tmp2[:], rhe[:], AL.mult)
            nc.vector.tensor_scalar(btgt[:, :, 1], tmp2[:], 10.0, None,
                                    AL.mult)
            rw0 = tp.tile([_P, SC], f32, tag="rw0")
            nc.vector.reciprocal(rw0[:], aw[:])
            rat = tp.tile([_P, SC], f32, tag="rat")
            nc.vector.tensor_tensor(rat[:], gw[:], rw0[:], AL.mult)
            lnw = tp.tile([_P, SC], f32, tag="lnw")
            nc.scalar.activation(lnw[:], rat[:], ACT.Ln)
            nc.vector.tensor_scalar(btgt[:, :, 2], lnw[:], 5.0, None, AL.mult)
            rh0 = tp.tile([_P, SC], f32, tag="rh0")
            nc.vector.reciprocal(rh0[:], ah[:])
            rat2 = tp.tile([_P, SC], f32, tag="rat2")
            nc.vector.tensor_tensor(rat2[:], gh[:], rh0[:], AL.mult)
            lnh = tp.tile([_P, SC], f32, tag="lnh")
            nc.scalar.activation(lnh[:], rat2[:], ACT.Ln)
            nc.vector.tensor_scalar(btgt[:, :, 3], lnh[:], 5.0, None, AL.mult)

            br_t = cp.tile([_P, SC, 4], f32)
            nc.sync.dma_start(br_t[:].rearrange("p sc c -> p (sc c)"), brt[:])
            dte = tp.tile([_P, SC, 4], f32, tag="dte")
            nc.vector.tensor_tensor(dte[:], btgt[:], br_t[:], AL.subtract)
            da = tp.tile([_P, SC, 4], f32, tag="da")
            nc.scalar.activation(da[:], dte[:], ACT.Abs)
            h1 = tp.tile([_P, SC, 4], f32, tag="h1")
            nc.vector.tensor_scalar(h1[:], da[:], 0.5, None, AL.mult)
            nc.vector.tensor_tensor(h1[:], h1[:], da[:], AL.mult)
            lin = tp.tile([_P, SC, 4], f32, tag="lin")
            nc.vector.tensor_scalar(lin[:], da[:], -0.5, None, AL.add)
            cmp = tp.tile([_P, SC, 4], mybir.dt.int32, tag="cmp")
            nc.vector.tensor_scalar(cmp[:], da[:], 1.0, None, AL.is_lt)
            nc.vector.copy_predicated(lin[:], cmp[:], h1[:])
            rv_t = cp.tile([_P, SC], f32)
            nc.sync.dma_start(rv_t[:], rvt[:])
            rvb = rv_t[:].rearrange("p (sc o) -> p sc o", o=1).broadcast_to(
                [_P, SC, 4])
            nc.vector.tensor_tensor(lin[:], lin[:], rvb, AL.mult)
            red8 = cp.tile([_P, 6], f32)
            bred = tp.tile([_P, SC], f32, tag="bred")
            nc.vector.tensor_reduce(bred[:], lin[:],
                                    axis=mybir.AxisListType.X, op=AL.add)
            nc.vector.tensor_reduce(
                red8[:, 0:2], bred[:].rearrange("p (s c) -> p s c", s=S),
                axis=mybir.AxisListType.X, op=AL.add)

            # ---- wing landmark loss
            evb0 = cp.tile([1, 3 * R], f32)
            nc.sync.dma_start(evb0[:], ev_c[:])
            evb = cp.tile([_P, 3 * R], f32)
            nc.gpsimd.partition_broadcast(evb[:], evb0[:])

            def ebr(q):  # const row q -> [P, SC, R] broadcast
                return evb[:, q * R:(q + 1) * R].rearrange(
                    "p (o d) -> p o d", o=1).broadcast_to([_P, SC, R])

            def rb(t):  # [P, SC] -> [P, SC, R]
                return t[:].rearrange("p (sc o) -> p sc o", o=1).broadcast_to(
                    [_P, SC, R])

            ctr = tp.tile([_P, SC, R], f32, tag="wt", bufs=10, name="ctr")
            nc.vector.tensor_tensor(ctr[:], rb(acx), ebr(0), AL.mult)
            c2 = tp.tile([_P, SC, R], f32, tag="wt", bufs=10, name="c2")
            nc.vector.tensor_tensor(c2[:], rb(acy), ebr(1), AL.mult)
            nc.vector.tensor_tensor(ctr[:], ctr[:], c2[:], AL.add)
            den = tp.tile([_P, SC, R], f32, tag="wt", bufs=10, name="den")
            nc.vector.tensor_tensor(den[:], rb(aw), ebr(0), AL.mult)
            d2 = tp.tile([_P, SC, R], f32, tag="wt", bufs=10, name="d2")
            nc.vector.tensor_tensor(d2[:], rb(ah), ebr(1), AL.mult)
            nc.vector.tensor_tensor(den[:], den[:], d2[:], AL.add)
            nc.vector.tensor_scalar(den[:], den[:], 1e-14, None, AL.add)
            rden = tp.tile([_P, SC, R], f32, tag="wt", bufs=10, name="rden")
            nc.vector.reciprocal(rden[:], den[:])
            ltg = tp.tile([_P, SC, R], f32, tag="wt", bufs=10, name="ltg")
            nc.vector.tensor_tensor(ltg[:], gt_t[:, :, 4:200], ctr[:],
                                    AL.subtract)
            nc.vector.tensor_tensor(ltg[:], ltg[:], rden[:], AL.mult)
            nc.vector.tensor_scalar(ltg[:], ltg[:], 10.0, None, AL.mult)
            lr16 = cp.tile([_P, SC, R], f16)
            nc.sync.dma_start(lr16[:].rearrange("p sc d -> p (sc d)"), lrh[:])
            lrf = tp.tile([_P, SC, R], f32, tag="wt", bufs=10, name="lrf")
            nc.vector.tensor_copy(lrf[:], lr16[:])
            dd = tp.tile([_P, SC, R], f32, tag="wt", bufs=10, name="dd")
            nc.vector.tensor_tensor(dd[:], ltg[:], lrf[:], AL.subtract)
            dda = tp.tile([_P, SC, R], f32, tag="wt", bufs=10, name="dda")
            nc.scalar.activation(dda[:], dd[:], ACT.Abs)
            nc.vector.tensor_tensor(dda[:], dda[:], ebr(2), AL.mult)
            ln3 = tp.tile([_P, SC, R], f32, tag="wt", bufs=10, name="ln3")
            nc.scalar.activation(ln3[:], dda[:], ACT.Ln, bias=1.0,
                                 scale=1.0 / _EPS)
            nc.vector.tensor_scalar(ln3[:], ln3[:], _OMEGA, None, AL.mult)
            lin2 = tp.tile([_P, SC, R], f32, tag="wt", bufs=10, name="lin2")
            nc.vector.tensor_scalar(lin2[:], dda[:], -_WING_C, None, AL.add)
            cmp2 = tp.tile([_P, SC, R], mybir.dt.int32, tag="wti", bufs=2,
                           name="cmp2")
            nc.vector.tensor_scalar(cmp2[:], dda[:], _OMEGA, None, AL.is_lt)
            nc.vector.copy_predicated(lin2[:], cmp2[:], ln3[:])
            lp_t = cp.tile([_P, SC], f32)
            nc.sync.dma_start(lp_t[:], lpt[:])
            nc.vector.tensor_tensor(lin2[:], lin2[:], rb(lp_t), AL.mult)
            wred = tp.tile([_P, SC], f32, tag="wred")
            nc.vector.tensor_reduce(wred[:], lin2[:],
                                    axis=mybir.AxisListType.X, op=AL.add)
            nc.vector.tensor_reduce(
                red8[:, 2:4], wred[:].rearrange("p (s c) -> p s c", s=S),
                axis=mybir.AxisListType.X, op=AL.add)

            # ---- pos_mean partials
            c0_t = cp.tile([_P, SC], f32)
            nc.sync.dma_start(c0_t[:], c0t[:])
            for s in range(S):
                csc = tp.tile([_P, C], f32, tag="csc")
                nc.vector.tensor_scalar(csc[:], c0_t[:, s * C:(s + 1) * C],
                                        -1.0, None, AL.mult, AL.add,
                                        accum_out=red8[:, 4 + s:5 + s])

            from concourse import bass_isa as bi
            red8r = cp.tile([_P, 6], f32)
            nc.gpsimd.partition_all_reduce(red8r[:], red8[:], channels=_P,
                                           reduce_op=bi.ReduceOp.add)

            sv0 = cp.tile([1, 16], f32)
            nc.sync.dma_start(sv0[:], svec[:])
            svb = cp.tile([_P, 16], f32)
            nc.gpsimd.partition_broadcast(svb[:], sv0[:])
            # svec: [i0_0, i0_1, i1_0, i1_1, nm0, nm1, gate0, gate1,
            #        invnp0, invnp1, ...]
            bbox = cp.tile([_P, S], f32)
            nc.vector.tensor_tensor(bbox[:], red8r[:, 0:2], svb[:, 0:2],
                                    AL.mult)
            ldm = cp.tile([_P, S], f32)
            nc.vector.tensor_tensor(ldm[:], red8r[:, 2:4], svb[:, 2:4],
                                    AL.mult)
            pm = tp.tile([_P, S], f32, tag="pm")
            nc.vector.tensor_tensor(pm[:], red8r[:, 4:6], svb[:, 8:10],
                                    AL.mult)
            nc.vector.tensor_tensor(pm[:], pm[:], svb[:, 4:6], AL.add)
            cls = cp.tile([_P, S], f32)
            nc.vector.tensor_tensor(cls[:], pm[:], svb[:, 6:8], AL.mult)
            nc.sync.dma_start(out[:, 0:2], cls[0:1, :])
            nc.sync.dma_start(out[:, 2:4], bbox[0:1, :])
            nc.sync.dma_start(out[:, 4:6], ldm[0:1, :])
    nc.compile()
    return nc


# --------------------------------------------------------------------------
# Runner (cached jit over 8 cores; mirrors bass2jax.run_bass_via_pjrt)
# --------------------------------------------------------------------------

class _Runner:
    def __init__(self, nc, n_cores=_NC):
        import jax
        import concourse.mybir as mybir
        from concourse import bass2jax
        from jax.sharding import Mesh, PartitionSpec
        from jax.experimental.shard_map import shard_map

        bass2jax.install_neuronx_cc_hook()
        self.nc = nc
        self.n_cores = n_cores
        partition_name = (nc.partition_id_tensor.name
                          if nc.partition_id_tensor else None)
        in_names, out_names, out_avals, zero_shapes = [], [], [], []
        for alloc in nc.m.functions[0].allocations:
            if not isinstance(alloc, mybir.MemoryLocationSet):
                continue
            name = alloc.memorylocations[0].name
            if alloc.kind == "ExternalInput":
                if name != partition_name:
                    in_names.append(name)
            elif alloc.kind == "ExternalOutput":
                shape = tuple(alloc.tensor_shape)
                dtype = mybir.dt.np(alloc.dtype)
                out_names.append(name)
                out_avals.append(jax.core.ShapedArray(shape, dtype))
                zero_shapes.append((shape, dtype))
        self.in_names = list(in_names)
        self.out_names = out_names
        self.out_avals = out_avals
        self.zero_shapes = zero_shapes
        n_params = len(in_names)
        n_outs = len(out_names)
        all_in = in_names + out_names
        if partition_name is not None:
            all_in = all_in + [partition_name]

        def _body(*args):
            operands = list(args)
            if partition_name is not None:
                operands.append(bass2jax.partition_id_tensor())
            outs = bass2jax._bass_exec_p.bind(
                *operands,
                out_avals=tuple(out_avals),
                in_names=tuple(all_in),
                out_names=tuple(out_names),
                lowering_input_output_aliases=(),
                sim_require_finite=True,
                sim_require_nnan=True,
                nc=nc,
            )
            return tuple(outs)

        devices = jax.devices()[:n_cores]
        mesh = Mesh(np.asarray(devices), ("core",))
        in_specs = (PartitionSpec("core"),) * (n_params + n_outs)
        out_specs = (PartitionSpec("core"),) * n_outs
        donate = tuple(range(n_params, n_params + n_outs))
        self._jit = jax.jit(
            shard_map(_body, mesh=mesh, in_specs=in_specs,
                      out_specs=out_specs, check_rep=False),
            donate_argnums=donate, keep_unused=True)

    def __call__(self, per_core_maps):
        concat = [
            np.concatenate([np.asarray(per_core_maps[c][n])
                            for c in range(self.n_cores)], axis=0)
            for n in self.in_names
        ]
        zeros = [np.zeros((self.n_cores * s[0],) + tuple(s[1:]), d)
                 for (s, d) in self.zero_shapes]
        outs = self._jit(*concat, *zeros)
        res = []
        for c in range(self.n_cores):
            m = {}
            for i, n in enumerate(self.out_names):
                a = np.asarray(outs[i])
                sh = self.out_avals[i].shape
                m[n] = a.reshape((self.n_cores,) + sh)[c]
            res.append(m)
        return res


# --------------------------------------------------------------------------
# Host glue
# --------------------------------------------------------------------------

def _prep_a(cls_h, ann_h):
    """Per-core phase-A input maps + per-sample box table."""
    cls1 = cls_h[:, :, 1].astype(np.float16).reshape(_B, _P, _NA)
    maps = []
    for c in range(_NC):
        binfo = np.zeros((1, _SPC * _N * 8), np.float32)
        for s in range(_SPC):
            b = c * _SPC + s
            boxes = ann_h[b, :, :4]
            valid = ann_h[b, :, 0] > 0
            barea = (boxes[:, 2] - boxes[:, 0]) * (boxes[:, 3] - boxes[:, 1])
            barea = np.where(valid, barea, 1e30).astype(np.float32)
            blk = binfo[0].reshape(_SPC, _N, 8)
            blk[s, :, 0:4] = boxes
            blk[s, :, 4] = barea
        maps.append({
            "cls1h": np.ascontiguousarray(cls1[c * _SPC:(c + 1) * _SPC]),
            "binfo": binfo,
        })
    return maps


def _prep_b(res_a, cls_h, breg_h, lreg_h, anc_full, ann_h):
    """Decode phase-A output, compact positives, build phase-B inputs."""
    S, C, R = _SPC, _K // _P, 196
    SC = S * C
    maps = []
    meta = []
    for c in range(_NC):
        packed = res_a[c]["packed"].reshape(_SPC, _A)
        stats = res_a[c]["stats"][0]
        lr = np.zeros((_P, SC, R), np.float16)
        br = np.zeros((_P, SC, 4), np.float32)
        ab = np.zeros((_P, SC, 4), np.float32)
        ab[:, :, 2:] = 1.0
        c0 = np.zeros((_P, SC), np.float32)
        rv = np.zeros((_P, SC), np.float32)
        lp = np.zeros((_P, SC), np.float32)
        ag = np.zeros((1, SC * _P), np.float32)
        sv = np.zeros((1, 16), np.float32)
        cmeta = []
        for s in range(_SPC):
            b = c * _SPC + s
            pos = packed[s] >= 128
            idx = np.nonzero(pos)[0]
            npos = idx.size
            overflow = npos > _K
            if overflow:
                idx = idx[:_K]
            n = idx.size
            argv = (packed[s, idx] & 127).astype(np.int64)
            hasldm = ann_h[b, :, 4:].sum(axis=1) > 0
            # row r -> (chunk, partition) = (r // P, r % P)
            ch = np.arange(n) // _P
            pa = np.arange(n) % _P
            sc = s * C + ch
            lr[pa, sc] = lreg_h[b, idx].astype(np.float16)
            br[pa, sc] = breg_h[b, idx]
            ab[pa, sc] = anc_full[idx]
            c0[pa, sc] = cls_h[b, idx, 0]
            rv[pa, sc] = 1.0
            lpv = hasldm[argv]
            lp[pa, sc] = lpv.astype(np.float32)
            agr = ag[0].reshape(SC, _P)
            agr[s * C:(s + 1) * C, :] = 0.0
            agr[sc, pa] = argv.astype(np.float32)
            nl = int(lpv.sum())
            has_gt = bool((ann_h[b, :, 0] > 0).any())
            gate = 1.0 if (has_gt and npos > 0) else 0.0
            lgate = 1.0 if (has_gt and nl > 0) else 0.0
            sv[0, 0 + s] = gate / (max(npos, 1) * 4.0)
            sv[0, 2 + s] = lgate / (max(nl, 1) * 196.0)
            sv[0, 4 + s] = stats[s]           # neg_mean from device
            sv[0, 6 + s] = gate
            sv[0, 8 + s] = 1.0 / max(npos, 1)
            cmeta.append({"npos": npos, "overflow": overflow, "b": b})
        maps.append({
            "lrh": np.ascontiguousarray(lr.reshape(_P, SC * R)),
            "brt": np.ascontiguousarray(br.reshape(_P, SC * 4)),
            "abt": np.ascontiguousarray(ab.reshape(_P, SC * 4)),
            "c0t": c0, "rvt": rv, "lpt": lp, "agt": ag,
            "ann2": np.ascontiguousarray(ann_h[c * _SPC:(c + 1) * _SPC]),
            "svec": sv,
        })
        meta.append(cmeta)
    return maps, meta


def _get_runners(anchors_np):
    key = hash(anchors_np.tobytes())
    if _state.get("key") == key:
        return _state["ra"], _state["rb"]
    nca = _build_phase_a(anchors_np)
    ncb = _build_phase_b()
    ra, rb = _Runner(nca), _Runner(ncb)
    _state.update(key=key, ra=ra, rb=rb, nca=nca, ncb=ncb)
    return ra, rb


def kernel(classifications, bbox_regressions, ldm_regressions, anchors,
           annotations):
    cls_h = np.asarray(classifications, np.float32)
    breg_h = np.asarray(bbox_regressions, np.float32)
    lreg_h = np.asarray(ldm_regressions, np.float32)
    anc_full = np.asarray(anchors, np.float32)[0]
    ann_h = np.asarray(annotations, np.float32)

    ra, rb = _get_runners(anc_full)
    maps_a = _prep_a(cls_h, ann_h)
    res_a = ra(maps_a)
    maps_b, meta = _prep_b(res_a, cls_h, breg_h, lreg_h, anc_full, ann_h)
    res_b = rb(maps_b)

    cls_out = np.zeros(_B, np.float32)
    bbox_out = np.zeros(_B, np.float32)
    ldm_out = np.zeros(_B, np.float32)
    for c in range(_NC):
        o = res_b[c]["out"][0]
        for s in range(_SPC):
            b = c * _SPC + s
            if meta[c][s]["overflow"]:
                cls_out[b], bbox_out[b], ldm_out[b] = _np_sample(
                    cls_h[b], breg_h[b], lreg_h[b], anc_full, ann_h[b])
            else:
                cls_out[b] = o[0 + s]
                bbox_out[b] = o[2 + s]
                ldm_out[b] = o[4 + s]
    return cls_out, bbox_out, ldm_out


def _np_sample(cls, breg, lreg, anchor, ann):
    """Exact numpy fallback for a single sample (npos > K overflow)."""
    valid = ann[:, 0] > 0
    boxes = ann[:, :4]
    ldm_ann = ann[:, 4:]
    has_gt = bool(valid.any())
    area = (boxes[:, 2] - boxes[:, 0]) * (boxes[:, 3] - boxes[:, 1])
    iw = np.minimum(anchor[:, 2][:, None], boxes[None, :, 2]) - \
        np.maximum(anchor[:, 0][:, None], boxes[None, :, 0])
    ih = np.minimum(anchor[:, 3][:, None], boxes[None, :, 3]) - \
        np.maximum(anchor[:, 1][:, None], boxes[None, :, 1])
    iw = np.clip(iw, 0.0, None)
    ih = np.clip(ih, 0.0, None)
    aw = anchor[:, 2] - anchor[:, 0]
    ah = anchor[:, 3] - anchor[:, 1]
    ua = np.clip((aw * ah)[:, None] + area[None, :] - iw * ih, 1e-8, None)
    iou = np.where(valid[None, :], iw * ih / ua, -1.0)
    iou_max = iou.max(axis=1)
    iou_arg = iou.argmax(axis=1)
    negm = iou_max < 0.4
    posm = iou_max >= 0.7
    npos = int(posm.sum())
    keep = min(int(negm.sum()), 3 * npos)
    neg_losses = np.where(negm, -cls[:, 1], -np.inf)
    srt = np.sort(neg_losses)[::-1]
    neg_mean = srt[:keep].sum(dtype=np.float32) / max(keep, 1)
    pos_mean = np.where(posm, -cls[:, 0], 0.0).sum(dtype=np.float32) / \
        max(npos, 1)
    cl = (pos_mean + neg_mean) if (has_gt and npos > 0) else 0.0
    gb = boxes[iou_arg]
    gw = gb[:, 2] - gb[:, 0]
    gh = gb[:, 3] - gb[:, 1]
    gcx = gb[:, 0] + 0.5 * gw
    gcy = gb[:, 1] + 0.5 * gh
    acx = anchor[:, 0] + 0.5 * aw
    acy = anchor[:, 1] + 0.5 * ah
    with np.errstate(all='ignore'):
        btgt = np.stack([(gcx - acx) / (aw + 1e-14) * 10,
                         (gcy - acy) / (ah + 1e-14) * 10,
                         np.log(gw / aw) * 5, np.log(gh / ah) * 5], axis=1)
    d = np.abs(btgt - breg)
    sl1 = np.where(d < 1.0, 0.5 * d * d, d - 0.5)
    bl = np.where(posm[:, None], sl1, 0.0).sum(dtype=np.float32) / \
        (max(npos, 1) * 4) if (has_gt and npos > 0) else 0.0
    gl = ldm_ann[iou_arg]
    lpos = posm & (gl.sum(axis=1) > 0)
    nl = int(lpos.sum())
    even = (np.arange(196) % 2) == 0
    ctr = np.where(even[None, :], acx[:, None], acy[:, None])
    den = np.where(even[None, :], aw[:, None], ah[:, None]) + 1e-14
    s = np.concatenate([np.ones(68, np.float32), 3 * np.ones(128, np.float32)])
    dd = np.abs((gl - ctr) / den / 0.1 * s - lreg * s)
    wing = np.where(dd < _OMEGA, _OMEGA * np.log1p(dd / _EPS), dd - _WING_C)
    ll = np.where(lpos[:, None], wing, 0.0).sum(dtype=np.float32) / \
        (max(nl, 1) * 196) if (has_gt and nl > 0) else 0.0
    return np.float32(cl), np.float32(bl), np.float32(ll)


# revision 12
# speedup vs baseline: 3.4672x; 3.4672x over previous
"""RetinaFace-style multi-task loss on 8 Trainium2 NeuronCores via Bass/Tile.

Data-parallel: 16 samples sharded 2-per-core across 8 cores. Two device
kernels per call:

  Phase A (Bass, dense): per sample, IoU of 102400 anchors x 32 GT boxes,
    running max + argmax, pos/neg thresholds, hard-negative-mined
    classification neg_mean via 5-round 16-way threshold search.
    Anchor tensor is embedded in the NEFF as a const (loaded to HBM once
    at model load; never re-shipped). Exports a packed uint8 plane
    (argmax | pos<<7) + per-sample stats.
  Host: decode packed plane, compact positive-anchor indices, slice the
    needed rows of bbox/ldm regressions (so the 1.25 GB ldm tensor never
    crosses the host-device link).
  Phase B (Bass, sparse): <=256 positive rows per sample. GT boxes and
    landmarks gathered from annotations on-device via one-hot matmul on
    the PE. SmoothL1 bbox loss + wing landmark loss + final combine.

Anchor layout on device: anchor a = p*800 + n  (p = SBUF partition,
n = free-dim column), so [102400, X] DRAM tensors reshape to [128, 800, X]
with fully contiguous per-partition DMA rows.
"""
import sys
import types
import numpy as np

_B, _A, _N = 16, 102400, 32
_NC = 8
_SPC = 2          # samples per core
_P, _NA = 128, 800  # A = P * NA
_K = 256          # max positives per sample handled on the sparse path
_OMEGA, _EPS = 3.0, 2.0
_WING_C = _OMEGA - _OMEGA * float(np.log(1.0 + _OMEGA / _EPS))
_VBIG = -100.0    # "minus infinity" for non-negative-anchor scores
_HI0 = 64.0       # upper bound for threshold search
_NITER = 5        # threshold-search rounds

_state = {}


# --------------------------------------------------------------------------
# Bass module builders
# --------------------------------------------------------------------------

def _build_phase_a(anchors_np):
    import concourse.tile as tile
    import concourse.mybir as mybir
    from concourse import bacc
    from concourse import bass_isa

    AL = mybir.AluOpType
    ACT = mybir.ActivationFunctionType
    f32 = mybir.dt.float32
    f16 = mybir.dt.float16
    u8 = mybir.dt.uint8
    i32 = mybir.dt.int32

    nc = bacc.Bacc("TRN2", target_bir_lowering=False, debug=False,
                   num_devices=_NC, name="loss_phase_a")
    cls1h = nc.dram_tensor("cls1h", [_SPC, _P, _NA], f16, kind="ExternalInput")
    binfo = nc.dram_tensor("binfo", [1, _SPC * _N * 8], f32, kind="ExternalInput")
    packed = nc.dram_tensor("packed", [_SPC, _P, _NA], u8, kind="ExternalOutput")
    stats = nc.dram_tensor("stats", [1, 8], f32, kind="ExternalOutput")

    anc_c = nc.inline_tensor(
        np.ascontiguousarray(anchors_np.reshape(_P, _NA * 4)), name="anc_c")
    coef_c = nc.inline_tensor(
        ((np.arange(16, dtype=np.float32) + 1.0) / 17.0).reshape(1, 16),
        name="coef_c")

    S = _SPC
    T24 = 24  # per-partition top-K kept for hard-negative mining
    with tile.TileContext(nc) as tc:
        with tc.tile_pool(name="keep", bufs=1) as cp, \
             tc.tile_pool(name="tmp", bufs=3) as tp, \
             tc.tile_pool(name="scr", bufs=4) as sp:
            def jt(nm, bufs=16):
                return tp.tile([_P, _NA], f32, tag="jt", name=nm, bufs=bufs)

            # ---- anchors -> contiguous planes
            anc_t = cp.tile([_P, _NA * 4], f32)
            nc.sync.dma_start(anc_t[:], anc_c[:])
            ancv = anc_t[:].rearrange("p (n c) -> p c n", c=4)
            ax = [cp.tile([_P, _NA], f32, tag=f"ax{q}", name=f"ax{q}")
                  for q in range(4)]
            for q in range(4):
                nc.vector.tensor_copy(ax[q][:], ancv[:, q, :])
            aw_t = jt("aw_t")
            nc.vector.tensor_tensor(aw_t[:], ax[2][:], ax[0][:], AL.subtract)
            ah_t = jt("ah_t")
            nc.vector.tensor_tensor(ah_t[:], ax[3][:], ax[1][:], AL.subtract)
            aarea = cp.tile([_P, _NA], f32)
            nc.vector.tensor_tensor(aarea[:], aw_t[:], ah_t[:], AL.mult)

            # ---- cls1 (fp16) -> negated fp32
            cls1_t = cp.tile([_P, S, _NA], f16)
            for s in range(S):
                nc.sync.dma_start(cls1_t[:, s, :], cls1h[s])
            vminus = cp.tile([_P, S, _NA], f32)
            nc.vector.tensor_scalar(vminus[:], cls1_t[:], -1.0, None, AL.mult)

            # ---- per-(s, j) box scalars, broadcast to all partitions
            b0 = cp.tile([1, S * _N * 8], f32)
            nc.sync.dma_start(b0[:], binfo[:])
            bb = cp.tile([_P, S * _N * 8], f32)
            nc.gpsimd.partition_broadcast(bb[:], b0[:])
            bv = bb[:].rearrange("p (s j q) -> p s j q", s=S, j=_N)

            def bq(s, j, q):  # [P, 1] per-partition scalar AP
                return bv[:, s, j, q:q + 1]

            # ---- IoU loop: running max of inter/ua as exact num/den pair
            num = [cp.tile([_P, _NA], f32, name=f"num{s}") for s in range(S)]
            den = [cp.tile([_P, _NA], f32, name=f"den{s}") for s in range(S)]
            arg = [cp.tile([_P, _NA], f32, name=f"arg{s}") for s in range(S)]
            for s in range(S):
                nc.vector.memset(num[s][:], -1.0)
                nc.vector.memset(den[s][:], 1.0)
                nc.vector.memset(arg[s][:], 0.0)

            for j in range(_N):
                for s in range(S):
                    # ts ops on DVE (2x mode); sample-1 plain TT on POOL;
                    # area add + relu on ACT
                    e = nc.gpsimd if s == 1 else nc.vector
                    t1 = jt("t1")
                    nc.vector.tensor_scalar(t1[:], ax[2][:], bq(s, j, 2),
                                            None, AL.min)
                    t2 = jt("t2")
                    nc.vector.tensor_scalar(t2[:], ax[0][:], bq(s, j, 0),
                                            None, AL.max)
                    t3 = jt("t3")
                    nc.vector.tensor_scalar(t3[:], ax[3][:], bq(s, j, 3),
                                            None, AL.min)
                    t4 = jt("t4")
                    nc.vector.tensor_scalar(t4[:], ax[1][:], bq(s, j, 1),
                                            None, AL.max)
                    iw = jt("iw")
                    e.tensor_tensor(iw[:], t1[:], t2[:], AL.subtract)
                    ih = jt("ih")
                    e.tensor_tensor(ih[:], t3[:], t4[:], AL.subtract)
                    ihc = jt("ihc")
                    nc.scalar.activation(ihc[:], ih[:], ACT.Relu)
                    u0 = jt("u0")
                    nc.scalar.activation(u0[:], aarea[:], ACT.Identity,
                                         bias=bq(s, j, 4))
                    inter = jt("inter")
                    nc.vector.scalar_tensor_tensor(inter[:], iw[:], 0.0,
                                                   ihc[:], AL.max, AL.mult)
                    u1 = jt("u1")
                    e.tensor_tensor(u1[:], u0[:], inter[:], AL.subtract)
                    ua = jt("ua")
                    nc.vector.tensor_scalar(ua[:], u1[:], 1e-8, None, AL.max)
                    gn = jt("gn")
                    e.tensor_tensor(gn[:], inter[:], den[s][:], AL.mult)
                    gd = jt("gd")
                    e.tensor_tensor(gd[:], num[s][:], ua[:], AL.mult)
                    gtf = jt("gtf")
                    nc.vector.tensor_tensor(gtf[:], gn[:], gd[:], AL.is_gt)
                    gti = tp.tile([_P, _NA], i32, tag="gti", name="gti",
                                  bufs=4)
                    nc.vector.tensor_copy(gti[:], gtf[:])
                    nc.vector.copy_predicated(num[s][:], gti[:], inter[:])
                    nc.vector.copy_predicated(den[s][:], gti[:], ua[:])
                    nc.vector.scalar_tensor_tensor(arg[s][:], gtf[:],
                                                   float(j), arg[s][:],
                                                   AL.mult, AL.max)

            # ---- pos / neg masks + counts (exact: num >= thr * den)
            cnt = cp.tile([_P, 4], f32)
            pos = cp.tile([_P, S, _NA], f32)
            neg = cp.tile([_P, S, _NA], f32)
            for s in range(S):
                th = sp.tile([_P, _NA], f32, tag="th", bufs=2, name="th")
                nc.vector.tensor_scalar(th[:], den[s][:], 0.7, None, AL.mult)
                thn = sp.tile([_P, _NA], f32, tag="thn", bufs=2, name="thn")
                nc.vector.tensor_scalar(thn[:], den[s][:], 0.4, None, AL.mult)
                posr = sp.tile([_P, _NA], f32, tag="posr", bufs=2, name="posr")
                nc.vector.tensor_tensor(posr[:], num[s][:], th[:], AL.is_ge)
                nc.vector.tensor_copy(pos[:, s, :], posr[:])
                nc.vector.tensor_reduce(cnt[:, s:s + 1], posr[:],
                                        axis=mybir.AxisListType.X, op=AL.add)
                negr = sp.tile([_P, _NA], f32, tag="negr", bufs=2, name="negr")
                nc.vector.tensor_tensor(negr[:], num[s][:], thn[:], AL.is_lt)
                nc.vector.tensor_copy(neg[:, s, :], negr[:])
                nc.vector.tensor_reduce(cnt[:, 2 + s:3 + s], negr[:],
                                        axis=mybir.AxisListType.X, op=AL.add)
            cntr = cp.tile([_P, 4], f32)
            nc.gpsimd.partition_all_reduce(cntr[:], cnt[:], channels=_P,
                                           reduce_op=bass_isa.ReduceOp.add)
            keep = cp.tile([_P, S], f32)
            nc.vector.scalar_tensor_tensor(keep[:], cntr[:, 0:2], 3.0,
                                           cntr[:, 2:4], AL.mult, AL.min)

            # ---- v = neg ? -cls1 : VBIG, then per-partition top-24
            v = cp.tile([_P, S, _NA], f32)
            nc.vector.memset(v[:], _VBIG)
            neg_i = cp.tile([_P, S, _NA], i32)
            nc.vector.tensor_copy(neg_i[:], neg[:])
            nc.vector.copy_predicated(v[:], neg_i[:], vminus[:])
            tops = cp.tile([_P, S, T24], f32)
            for s in range(S):
                vv = v[:, s, :]
                for r in range(T24 // 8):
                    nc.vector.max(tops[:, s, r * 8:(r + 1) * 8], vv)
                    if r < T24 // 8 - 1:
                        vn = sp.tile([_P, _NA], f32, tag="vmr", bufs=2,
                                     name="vmr")
                        nc.vector.match_replace(
                            vn[:], tops[:, s, r * 8:(r + 1) * 8], vv, -200.0)
                        vv = vn[:]

            # ---- threshold search on the compacted top values
            co0 = cp.tile([1, 16], f32)
            nc.sync.dma_start(co0[:], coef_c[:])
            cb = cp.tile([_P, 16], f32)
            nc.gpsimd.partition_broadcast(cb[:], co0[:])
            cbv = cb[:].rearrange("p (o k) -> p o k", o=1).broadcast_to(
                [_P, S, 16])
            p100 = cp.tile([_P, S, 16], f32)
            nc.vector.memset(p100[:], 100.0)
            lo = cp.tile([_P, S], f32)
            nc.vector.memset(lo[:], _VBIG)
            hi = cp.tile([_P, S], f32)
            nc.vector.memset(hi[:], _HI0)

            def b2(t):
                return t[:].rearrange("p (s o) -> p s o", o=1).broadcast_to(
                    [_P, S, 16])

            keep_b = b2(keep)
            for it in range(_NITER):
                d = tp.tile([_P, S], f32, tag="d")
                nc.vector.tensor_tensor(d[:], hi[:], lo[:], AL.subtract)
                tk = tp.tile([_P, S, 16], f32, tag="tk")
                nc.vector.tensor_tensor(tk[:], cbv, b2(d), AL.mult)
                nc.vector.tensor_tensor(tk[:], tk[:], b2(lo), AL.add)
                cnts = tp.tile([_P, S * 16], f32, tag="cnts")
                for s in range(S):
                    for k in range(16):
                        scr = sp.tile([_P, T24], f32, tag="scr")
                        nc.vector.tensor_scalar(
                            scr[:], tops[:, s, :], tk[:, s, k:k + 1], None,
                            AL.is_ge, AL.add,
                            accum_out=cnts[:, s * 16 + k:s * 16 + k + 1])
                cntsr = tp.tile([_P, S * 16], f32, tag="cntsr")
                nc.gpsimd.partition_all_reduce(
                    cntsr[:], cnts[:], channels=_P,
                    reduce_op=bass_isa.ReduceOp.add)
                cnv = cntsr[:].rearrange("p (s k) -> p s k", s=S)
                big = tp.tile([_P, S, 16], i32, tag="big")
                nc.vector.tensor_tensor(big[:], cnv, keep_b, AL.is_ge)
                bt = tp.tile([_P, S, 16], f32, tag="bt")
                nc.vector.memset(bt[:], _VBIG)
                nc.vector.copy_predicated(bt[:], big[:], tk[:])
                rmx = tp.tile([_P, S], f32, tag="rmx")
                nc.vector.tensor_reduce(rmx[:], bt[:],
                                        axis=mybir.AxisListType.X, op=AL.max)
                nc.vector.tensor_tensor(lo[:], lo[:], rmx[:], AL.max)
                ht = tp.tile([_P, S, 16], f32, tag="ht")
                nc.vector.tensor_copy(ht[:], tk[:])
                nc.vector.copy_predicated(ht[:], big[:], p100[:])
                rmn = tp.tile([_P, S], f32, tag="rmn")
                nc.vector.tensor_reduce(rmn[:], ht[:],
                                        axis=mybir.AxisListType.X, op=AL.min)
                nc.vector.tensor_tensor(hi[:], hi[:], rmn[:], AL.min)

            # ---- exact sum/count above lo (on compacted tops)
            sc4 = cp.tile([_P, 4], f32)
            for s in range(S):
                sel = sp.tile([_P, T24], f32, tag="sel", bufs=2)
                nc.vector.scalar_tensor_tensor(sel[:], tops[:, s, :],
                                               lo[:, s:s + 1], tops[:, s, :],
                                               AL.is_ge, AL.mult)
                nc.vector.tensor_reduce(sc4[:, s:s + 1], sel[:],
                                        axis=mybir.AxisListType.X, op=AL.add)
                scc = sp.tile([_P, T24], f32, tag="scc", bufs=2)
                nc.vector.tensor_scalar(scc[:], tops[:, s, :], lo[:, s:s + 1],
                                        None, AL.is_ge, AL.add,
                                        accum_out=sc4[:, 2 + s:3 + s])
            sc4r = cp.tile([_P, 4], f32)
            nc.gpsimd.partition_all_reduce(sc4r[:], sc4[:], channels=_P,
                                           reduce_op=bass_isa.ReduceOp.add)
            # neg_mean = (s_lo - (c_lo - keep) * lo) / max(keep, 1)
            e1 = tp.tile([_P, S], f32, tag="e1")
            nc.vector.tensor_tensor(e1[:], sc4r[:, 2:4], keep[:], AL.subtract)
            nc.vector.tensor_tensor(e1[:], e1[:], lo[:], AL.mult)
            nc.vector.tensor_tensor(e1[:], sc4r[:, 0:2], e1[:], AL.subtract)
            kf = tp.tile([_P, S], f32, tag="kf")
            nc.vector.tensor_scalar(kf[:], keep[:], 1.0, None, AL.max)
            rk = tp.tile([_P, S], f32, tag="rk")
            nc.vector.reciprocal(rk[:], kf[:])
            nm = cp.tile([_P, S], f32)
            nc.vector.tensor_tensor(nm[:], e1[:], rk[:], AL.mult)

            # ---- packed output
            pku = cp.tile([_P, S, _NA], u8)
            for s in range(S):
                pk = jt("pk")
                nc.vector.scalar_tensor_tensor(pk[:], pos[:, s, :], 128.0,
                                               arg[s][:], AL.mult, AL.add)
                nc.vector.tensor_copy(pku[:, s, :], pk[:])
                nc.sync.dma_start(packed[s], pku[:, s, :])
            nc.sync.dma_start(stats[:, 0:2], nm[0:1, :])
            nc.sync.dma_start(stats[:, 2:4], cntr[0:1, 0:2])
            nc.sync.dma_start(stats[:, 4:6], cntr[0:1, 2:4])
    nc.compile()
    return nc


def _build_phase_b():
    import concourse.tile as tile
    import concourse.mybir as mybir
    from concourse import bacc
    from concourse import bass_isa

    AL = mybir.AluOpType
    ACT = mybir.ActivationFunctionType
    f32 = mybir.dt.float32
    f16 = mybir.dt.float16

    S, C, R = _SPC, _K // _P, 196
    SC = S * C
    nc = bacc.Bacc("TRN2", target_bir_lowering=False, debug=False,
                   num_devices=_NC, name="loss_phase_b")
    lrh = nc.dram_tensor("lrh", [_P, SC * R], f16, kind="ExternalInput")
    brt = nc.dram_tensor("brt", [_P, SC * 4], f32, kind="ExternalInput")
    abt = nc.dram_tensor("abt", [_P, SC * 4], f32, kind="ExternalInput")
    c0t = nc.dram_tensor("c0t", [_P, SC], f32, kind="ExternalInput")
    rvt = nc.dram_tensor("rvt", [_P, SC], f32, kind="ExternalInput")
    lpt = nc.dram_tensor("lpt", [_P, SC], f32, kind="ExternalInput")
    agt = nc.dram_tensor("agt", [1, SC * _P], f32, kind="ExternalInput")
    ann2 = nc.dram_tensor("ann2", [S, _N, 200], f32, kind="ExternalInput")
    svec = nc.dram_tensor("svec", [1, 16], f32, kind="ExternalInput")
    out = nc.dram_tensor("out", [1, 8], f32, kind="ExternalOutput")

    ev = np.zeros((1, 3 * R), np.float32)
    ev[0, 0:R][0::2] = 1.0                      # even mask
    ev[0, R:2 * R][1::2] = 1.0                  # odd mask
    ev[0, 2 * R:2 * R + 68] = 1.0               # s vector
    ev[0, 2 * R + 68:3 * R] = 3.0
    ev_c = nc.inline_tensor(ev, name="ev_c")

    with tile.TileContext(nc) as tc:
        with tc.tile_pool(name="keep", bufs=1) as cp, \
             tc.tile_pool(name="tmp", bufs=3) as tp, \
             tc.tile_pool(name="ps", bufs=2, space="PSUM") as pp:
            # ---- on-device gather of GT boxes + landmarks via one-hot matmul
            annt = cp.tile([_N, S * 200], f32)
            for s in range(S):
                nc.sync.dma_start(annt[:, s * 200:(s + 1) * 200], ann2[s])
            ag0 = cp.tile([1, SC * _P], f32)
            nc.sync.dma_start(ag0[:], agt[:])
            agb = cp.tile([_N, SC * _P], f32)
            nc.gpsimd.partition_broadcast(agb[:], ag0[:], channels=_N)
            iot = cp.tile([_N, 1], mybir.dt.int32)
            nc.gpsimd.iota(iot[:], pattern=[[0, 1]], base=0,
                           channel_multiplier=1)
            iof = cp.tile([_N, 1], f32)
            nc.vector.tensor_copy(iof[:], iot[:])
            oh = cp.tile([_N, SC * _P], f32)
            nc.vector.tensor_tensor(oh[:], agb[:],
                                    iof[:].broadcast_to([_N, SC * _P]),
                                    AL.is_equal)
            gt_t = cp.tile([_P, SC, 200], f32)
            for s in range(S):
                for c in range(C):
                    sc = s * C + c
                    gps = pp.tile([_P, 200], f32, tag="gps")
                    nc.tensor.matmul(gps[:], oh[:, sc * _P:(sc + 1) * _P],
                                     annt[:, s * 200:(s + 1) * 200],
                                     start=True, stop=True)
                    nc.vector.tensor_copy(gt_t[:, sc, :], gps[:])

            # ---- anchor-row planes
            ab_t = cp.tile([_P, SC * 4], f32)
            nc.sync.dma_start(ab_t[:], abt[:])
            abv = ab_t[:].rearrange("p (sc c) -> p c sc", c=4)
            aX = [tp.tile([_P, SC], f32, tag=f"aX{q}", name=f"aX{q}")
                  for q in range(4)]
            for q in range(4):
                nc.vector.tensor_copy(aX[q][:], abv[:, q, :])
            aw = cp.tile([_P, SC], f32)
            nc.vector.tensor_tensor(aw[:], aX[2][:], aX[0][:], AL.subtract)
            ah = cp.tile([_P, SC], f32)
            nc.vector.tensor_tensor(ah[:], aX[3][:], aX[1][:], AL.subtract)
            acx = cp.tile([_P, SC], f32)
            nc.vector.scalar_tensor_tensor(acx[:], aw[:], 0.5, aX[0][:],
                                           AL.mult, AL.add)
            acy = cp.tile([_P, SC], f32)
            nc.vector.scalar_tensor_tensor(acy[:], ah[:], 0.5, aX[1][:],
                                           AL.mult, AL.add)
            gx = gt_t[:].rearrange("p sc d -> p d sc")  # strided views
            gw = cp.tile([_P, SC], f32)
            nc.vector.tensor_tensor(gw[:], gx[:, 2, :], gx[:, 0, :],
                                    AL.subtract)
            gh = cp.tile([_P, SC], f32)
            nc.vector.tensor_tensor(gh[:], gx[:, 3, :], gx[:, 1, :],
                                    AL.subtract)
            gcx = tp.tile([_P, SC], f32, tag="gcx")
            nc.vector.scalar_tensor_tensor(gcx[:], gw[:], 0.5, gx[:, 0, :],
                                           AL.mult, AL.add)
            gcy = tp.tile([_P, SC], f32, tag="gcy")
            nc.vector.scalar_tensor_tensor(gcy[:], gh[:], 0.5, gx[:, 1, :],
                                           AL.mult, AL.add)

            # ---- bbox targets & SmoothL1
            btgt = cp.tile([_P, SC, 4], f32)
            awe = tp.tile([_P, SC], f32, tag="awe")
            nc.vector.tensor_scalar(awe[:], aw[:], 1e-14, None, AL.add)
            rwe = tp.tile([_P, SC], f32, tag="rwe")
            nc.vector.reciprocal(rwe[:], awe[:])
            ahe = tp.tile([_P, SC], f32, tag="ahe")
            nc.vector.tensor_scalar(ahe[:], ah[:], 1e-14, None, AL.add)
            rhe = tp.tile([_P, SC], f32, tag="rhe")
            nc.vector.reciprocal(rhe[:], ahe[:])
            tmp1 = tp.tile([_P, SC], f32, tag="tmp1")
            nc.vector.tensor_tensor(tmp1[:], gcx[:], acx[:], AL.subtract)
            nc.vector.tensor_tensor(tmp1[:], tmp1[:], rwe[:], AL.mult)
            nc.vector.tensor_scalar(btgt[:, :, 0], tmp1[:], 10.0, None,
                                    AL.mult)
            tmp2 = tp.tile([_P, SC], f32, tag="tmp2")
            nc.vector.tensor_tensor(tmp2[:], gcy[:], acy[:], AL.subtract)
            nc.vector.tensor_tensor(tmp2[:], tmp2[:], rhe[:], AL.mult)
            nc.vector.tensor_scalar(btgt[:, :, 1], tmp2[:], 10.0, None,
                                    AL.mult)
            rw0 = tp.tile([_P, SC], f32, tag="rw0")
            nc.vector.reciprocal(rw0[:], aw[:])
            rat = tp.tile([_P, SC], f32, tag="rat")
            nc.vector.tensor_tensor(rat[:], gw[:], rw0[:], AL.mult)
            lnw = tp.tile([_P, SC], f32, tag="lnw")
            nc.scalar.activation(lnw[:], rat[:], ACT.Ln)
            nc.vector.tensor_scalar(btgt[:, :, 2], lnw[:], 5.0, None, AL.mult)
            rh0 = tp.tile([_P, SC], f32, tag="rh0")
            nc.vector.reciprocal(rh0[:], ah[:])
            rat2 = tp.tile([_P, SC], f32, tag="rat2")
            nc.vector.tensor_tensor(rat2[:], gh[:], rh0[:], AL.mult)
            lnh = tp.tile([_P, SC], f32, tag="lnh")
            nc.scalar.activation(lnh[:], rat2[:], ACT.Ln)
            nc.vector.tensor_scalar(btgt[:, :, 3], lnh[:], 5.0, None, AL.mult)

            br_t = cp.tile([_P, SC, 4], f32)
            nc.sync.dma_start(br_t[:].rearrange("p sc c -> p (sc c)"), brt[:])
            dte = tp.tile([_P, SC, 4], f32, tag="dte")
            nc.vector.tensor_tensor(dte[:], btgt[:], br_t[:], AL.subtract)
            da = tp.tile([_P, SC, 4], f32, tag="da")
            nc.scalar.activation(da[:], dte[:], ACT.Abs)
            h1 = tp.tile([_P, SC, 4], f32, tag="h1")
            nc.vector.tensor_scalar(h1[:], da[:], 0.5, None, AL.mult)
            nc.vector.tensor_tensor(h1[:], h1[:], da[:], AL.mult)
            lin = tp.tile([_P, SC, 4], f32, tag="lin")
            nc.vector.tensor_scalar(lin[:], da[:], -0.5, None, AL.add)
            cmp = tp.tile([_P, SC, 4], mybir.dt.int32, tag="cmp")
            nc.vector.tensor_scalar(cmp[:], da[:], 1.0, None, AL.is_lt)
            nc.vector.copy_predicated(lin[:], cmp[:], h1[:])
            rv_t = cp.tile([_P, SC], f32)
            nc.sync.dma_start(rv_t[:], rvt[:])
            rvb = rv_t[:].rearrange("p (sc o) -> p sc o", o=1).broadcast_to(
                [_P, SC, 4])
            nc.vector.tensor_tensor(lin[:], lin[:], rvb, AL.mult)
            red8 = cp.tile([_P, 6], f32)
            bred = tp.tile([_P, SC], f32, tag="bred")
            nc.vector.tensor_reduce(bred[:], lin[:],
                                    axis=mybir.AxisListType.X, op=AL.add)
            nc.vector.tensor_reduce(
                red8[:, 0:2], bred[:].rearrange("p (s c) -> p s c", s=S),
                axis=mybir.AxisListType.X, op=AL.add)

            # ---- wing landmark loss
            evb0 = cp.tile([1, 3 * R], f32)
            nc.sync.dma_start(evb0[:], ev_c[:])
            evb = cp.tile([_P, 3 * R], f32)
            nc.gpsimd.partition_broadcast(evb[:], evb0[:])

            def ebr(q):  # const row q -> [P, SC, R] broadcast
                return evb[:, q * R:(q + 1) * R].rearrange(
                    "p (o d) -> p o d", o=1).broadcast_to([_P, SC, R])

            def rb(t):  # [P, SC] -> [P, SC, R]
                return t[:].rearrange("p (sc o) -> p sc o", o=1).broadcast_to(
                    [_P, SC, R])

            ctr = tp.tile([_P, SC, R], f32, tag="wt", bufs=10, name="ctr")
            nc.vector.tensor_tensor(ctr[:], rb(acx), ebr(0), AL.mult)
            c2 = tp.tile([_P, SC, R], f32, tag="wt", bufs=10, name="c2")
            nc.vector.tensor_tensor(c2[:], rb(acy), ebr(1), AL.mult)
            nc.vector.tensor_tensor(ctr[:], ctr[:], c2[:], AL.add)
            den = tp.tile([_P, SC, R], f32, tag="wt", bufs=10, name="den")
            nc.vector.tensor_tensor(den[:], rb(aw), ebr(0), AL.mult)
            d2 = tp.tile([_P, SC, R], f32, tag="wt", bufs=10, name="d2")
            nc.vector.tensor_tensor(d2[:], rb(ah), ebr(1), AL.mult)
            nc.vector.tensor_tensor(den[:], den[:], d2[:], AL.add)
            nc.vector.tensor_scalar(den[:], den[:], 1e-14, None, AL.add)
            rden = tp.tile([_P, SC, R], f32, tag="wt", bufs=10, name="rden")
            nc.vector.reciprocal(rden[:], den[:])
            ltg = tp.tile([_P, SC, R], f32, tag="wt", bufs=10, name="ltg")
            nc.vector.tensor_tensor(ltg[:], gt_t[:, :, 4:200], ctr[:],
                                    AL.subtract)
            nc.vector.tensor_tensor(ltg[:], ltg[:], rden[:], AL.mult)
            nc.vector.tensor_scalar(ltg[:], ltg[:], 10.0, None, AL.mult)
            lr16 = cp.tile([_P, SC, R], f16)
            nc.sync.dma_start(lr16[:].rearrange("p sc d -> p (sc d)"), lrh[:])
            lrf = tp.tile([_P, SC, R], f32, tag="wt", bufs=10, name="lrf")
            nc.vector.tensor_copy(lrf[:], lr16[:])
            dd = tp.tile([_P, SC, R], f32, tag="wt", bufs=10, name="dd")
            nc.vector.tensor_tensor(dd[:], ltg[:], lrf[:], AL.subtract)
            dda = tp.tile([_P, SC, R], f32, tag="wt", bufs=10, name="dda")
            nc.scalar.activation(dda[:], dd[:], ACT.Abs)
            nc.vector.tensor_tensor(dda[:], dda[:], ebr(2), AL.mult)
            ln3 = tp.tile([_P, SC, R], f32, tag="wt", bufs=10, name="ln3")
            nc.scalar.activation(ln3[:], dda[:], ACT.Ln, bias=1.0,
                                 scale=1.0 / _EPS)
            nc.vector.tensor_scalar(ln3[:], ln3[:], _OMEGA, None, AL.mult)
            lin2 = tp.tile([_P, SC, R], f32, tag="wt", bufs=10, name="lin2")
            nc.vector.tensor_scalar(lin2[:], dda[:], -_WING_C, None, AL.add)
            cmp2 = tp.tile([_P, SC, R], mybir.dt.int32, tag="wti", bufs=2,
                           name="cmp2")
            nc.vector.tensor_scalar(cmp2[:], dda[:], _OMEGA, None, AL.is_lt)
            nc.vector.copy_predicated(lin2[:], cmp2[:], ln3[:])
            lp_t = cp.tile([_P, SC], f32)
            nc.sync.dma_start(lp_t[:], lpt[:])
            nc.vector.tensor_tensor(lin2[:], lin2[:], rb(lp_t), AL.mult)
            wred = tp.tile([_P, SC], f32, tag="wred")
            nc.vector.tensor_reduce(wred[:], lin2[:],
                                    axis=mybir.AxisListType.X, op=AL.add)
            nc.vector.tensor_reduce(
                red8[:, 2:4], wred[:].rearrange("p (s c) -> p s c", s=S),
                axis=mybir.AxisListType.X, op=AL.add)

            # ---- pos_mean partials
            c0_t = cp.tile([_P, SC], f32)
            nc.sync.dma_start(c0_t[:], c0t[:])
            for s in range(S):
                csc = tp.tile([_P, C], f32, tag="csc")
                nc.vector.tensor_scalar(csc[:], c0_t[:, s * C:(s + 1) * C],
                                        -1.0, None, AL.mult, AL.add,
                                        accum_out=red8[:, 4 + s:5 + s])

            from concourse import bass_isa as bi
            red8r = cp.tile([_P, 6], f32)
            nc.gpsimd.partition_all_reduce(red8r[:], red8[:], channels=_P,
                                           reduce_op=bi.ReduceOp.add)

            sv0 = cp.tile([1, 16], f32)
            nc.sync.dma_start(sv0[:], svec[:])
            svb = cp.tile([_P, 16], f32)
            nc.gpsimd.partition_broadcast(svb[:], sv0[:])
            # svec: [i0_0, i0_1, i1_0, i1_1, nm0, nm1, gate0, gate1,
            #        invnp0, invnp1, ...]
            bbox = cp.tile([_P, S], f32)
            nc.vector.tensor_tensor(bbox[:], red8r[:, 0:2], svb[:, 0:2],
                                    AL.mult)
            ldm = cp.tile([_P, S], f32)
            nc.vector.tensor_tensor(ldm[:], red8r[:, 2:4], svb[:, 2:4],
                                    AL.mult)
            pm = tp.tile([_P, S], f32, tag="pm")
            nc.vector.tensor_tensor(pm[:], red8r[:, 4:6], svb[:, 8:10],
                                    AL.mult)
            nc.vector.tensor_tensor(pm[:], pm[:], svb[:, 4:6], AL.add)
            cls = cp.tile([_P, S], f32)
            nc.vector.tensor_tensor(cls[:], pm[:], svb[:, 6:8], AL.mult)
            nc.sync.dma_start(out[:, 0:2], cls[0:1, :])
            nc.sync.dma_start(out[:, 2:4], bbox[0:1, :])
            nc.sync.dma_start(out[:, 4:6], ldm[0:1, :])
    nc.compile()
    return nc


# --------------------------------------------------------------------------
# Runner (cached jit over 8 cores; mirrors bass2jax.run_bass_via_pjrt)
# --------------------------------------------------------------------------

class _Runner:
    def __init__(self, nc, n_cores=_NC):
        import jax
        import concourse.mybir as mybir
        from concourse import bass2jax
        from jax.sharding import Mesh, PartitionSpec
        from jax.experimental.shard_map import shard_map

        bass2jax.install_neuronx_cc_hook()
        self.nc = nc
        self.n_cores = n_cores
        partition_name = (nc.partition_id_tensor.name
                          if nc.partition_id_tensor else None)
        in_names, out_names, out_avals, zero_shapes = [], [], [], []
        for alloc in nc.m.functions[0].allocations:
            if not isinstance(alloc, mybir.MemoryLocationSet):
                continue
            name = alloc.memorylocations[0].name
            if alloc.kind == "ExternalInput":
                if name != partition_name:
                    in_names.append(name)
            elif alloc.kind == "ExternalOutput":
                shape = tuple(alloc.tensor_shape)
                dtype = mybir.dt.np(alloc.dtype)
                out_names.append(name)
                out_avals.append(jax.core.ShapedArray(shape, dtype))
                zero_shapes.append((shape, dtype))
        self.in_names = list(in_names)
        self.out_names = out_names
        self.out_avals = out_avals
        self.zero_shapes = zero_shapes
        n_params = len(in_names)
        n_outs = len(out_names)
        all_in = in_names + out_names
        if partition_name is not None:
            all_in = all_in + [partition_name]

        def _body(*args):
            operands = list(args)
            if partition_name is not None:
                operands.append(bass2jax.partition_id_tensor())
            outs = bass2jax._bass_exec_p.bind(
                *operands,
                out_avals=tuple(out_avals),
                in_names=tuple(all_in),
                out_names=tuple(out_names),
                lowering_input_output_aliases=(),
                sim_require_finite=True,
                sim_require_nnan=True,
                nc=nc,
            )
            return tuple(outs)

        devices = jax.devices()[:n_cores]
        mesh = Mesh(np.asarray(devices), ("core",))
        in_specs = (PartitionSpec("core"),) * (n_params + n_outs)
        out_specs = (PartitionSpec("core"),) * n_outs
        donate = tuple(range(n_params, n_params + n_outs))
        self._jit = jax.jit(
            shard_map(_body, mesh=mesh, in_specs=in_specs,
                      out_specs=out_specs, check_rep=False),
            donate_argnums=donate, keep_unused=True)

    def __call__(self, per_core_maps):
        concat = [
            np.concatenate([np.asarray(per_core_maps[c][n])
                            for c in range(self.n_cores)], axis=0)
            for n in self.in_names
        ]
        zeros = [np.zeros((self.n_cores * s[0],) + tuple(s[1:]), d)
                 for (s, d) in self.zero_shapes]
        outs = self._jit(*concat, *zeros)
        res = []
        for c in range(self.n_cores):
            m = {}
            for i, n in enumerate(self.out_names):
                a = np.asarray(outs[i])
                sh = self.out_avals[i].shape
                m[n] = a.reshape((self.n_cores,) + sh)[c]
            res.append(m)
        return res


# --------------------------------------------------------------------------
# Host glue
# --------------------------------------------------------------------------

def _prep_a(cls_h, ann_h):
    """Per-core phase-A input maps + per-sample box table."""
    cls1 = cls_h[:, :, 1].astype(np.float16).reshape(_B, _P, _NA)
    maps = []
    for c in range(_NC):
        binfo = np.zeros((1, _SPC * _N * 8), np.float32)
        for s in range(_SPC):
            b = c * _SPC + s
            boxes = ann_h[b, :, :4]
            valid = ann_h[b, :, 0] > 0
            barea = (boxes[:, 2] - boxes[:, 0]) * (boxes[:, 3] - boxes[:, 1])
            barea = np.where(valid, barea, 1e30).astype(np.float32)
            blk = binfo[0].reshape(_SPC, _N, 8)
            blk[s, :, 0:4] = boxes
            blk[s, :, 4] = barea
        maps.append({
            "cls1h": np.ascontiguousarray(cls1[c * _SPC:(c + 1) * _SPC]),
            "binfo": binfo,
        })
    return maps


def _prep_b(res_a, cls_h, breg_h, lreg_h, anc_full, ann_h):
    """Decode phase-A output, compact positives, build phase-B inputs."""
    S, C, R = _SPC, _K // _P, 196
    SC = S * C
    maps = []
    meta = []
    for c in range(_NC):
        packed = res_a[c]["packed"].reshape(_SPC, _A)
        stats = res_a[c]["stats"][0]
        lr = np.zeros((_P, SC, R), np.float16)
        br = np.zeros((_P, SC, 4), np.float32)
        ab = np.zeros((_P, SC, 4), np.float32)
        ab[:, :, 2:] = 1.0
        c0 = np.zeros((_P, SC), np.float32)
        rv = np.zeros((_P, SC), np.float32)
        lp = np.zeros((_P, SC), np.float32)
        ag = np.zeros((1, SC * _P), np.float32)
        sv = np.zeros((1, 16), np.float32)
        cmeta = []
        for s in range(_SPC):
            b = c * _SPC + s
            pos = packed[s] >= 128
            idx = np.nonzero(pos)[0]
            npos = idx.size
            overflow = npos > _K
            if overflow:
                idx = idx[:_K]
            n = idx.size
            argv = (packed[s, idx] & 127).astype(np.int64)
            hasldm = ann_h[b, :, 4:].sum(axis=1) > 0
            # row r -> (chunk, partition) = (r // P, r % P)
            ch = np.arange(n) // _P
            pa = np.arange(n) % _P
            sc = s * C + ch
            lr[pa, sc] = lreg_h[b, idx].astype(np.float16)
            br[pa, sc] = breg_h[b, idx]
            ab[pa, sc] = anc_full[idx]
            c0[pa, sc] = cls_h[b, idx, 0]
            rv[pa, sc] = 1.0
            lpv = hasldm[argv]
            lp[pa, sc] = lpv.astype(np.float32)
            agr = ag[0].reshape(SC, _P)
            agr[s * C:(s + 1) * C, :] = 0.0
            agr[sc, pa] = argv.astype(np.float32)
            nl = int(lpv.sum())
            has_gt = bool((ann_h[b, :, 0] > 0).any())
            gate = 1.0 if (has_gt and npos > 0) else 0.0
            lgate = 1.0 if (has_gt and nl > 0) else 0.0
            sv[0, 0 + s] = gate / (max(npos, 1) * 4.0)
            sv[0, 2 + s] = lgate / (max(nl, 1) * 196.0)
            sv[0, 4 + s] = stats[s]           # neg_mean from device
            sv[0, 6 + s] = gate
            sv[0, 8 + s] = 1.0 / max(npos, 1)
            cmeta.append({"npos": npos, "overflow": overflow, "b": b})
        maps.append({
            "lrh": np.ascontiguousarray(lr.reshape(_P, SC * R)),
            "brt": np.ascontiguousarray(br.reshape(_P, SC * 4)),
            "abt": np.ascontiguousarray(ab.reshape(_P, SC * 4)),
            "c0t": c0, "rvt": rv, "lpt": lp, "agt": ag,
            "ann2": np.ascontiguousarray(ann_h[c * _SPC:(c + 1) * _SPC]),
            "svec": sv,
        })
        meta.append(cmeta)
    return maps, meta


def _get_runners(anchors_np):
    key = hash(anchors_np.tobytes())
    if _state.get("key") == key:
        return _state["ra"], _state["rb"]
    nca = _build_phase_a(anchors_np)
    ncb = _build_phase_b()
    ra, rb = _Runner(nca), _Runner(ncb)
    _state.update(key=key, ra=ra, rb=rb, nca=nca, ncb=ncb)
    return ra, rb


def kernel(classifications, bbox_regressions, ldm_regressions, anchors,
           annotations):
    cls_h = np.asarray(classifications, np.float32)
    breg_h = np.asarray(bbox_regressions, np.float32)
    lreg_h = np.asarray(ldm_regressions, np.float32)
    anc_full = np.asarray(anchors, np.float32)[0]
    ann_h = np.asarray(annotations, np.float32)

    ra, rb = _get_runners(anc_full)
    maps_a = _prep_a(cls_h, ann_h)
    res_a = ra(maps_a)
    maps_b, meta = _prep_b(res_a, cls_h, breg_h, lreg_h, anc_full, ann_h)
    res_b = rb(maps_b)

    cls_out = np.zeros(_B, np.float32)
    bbox_out = np.zeros(_B, np.float32)
    ldm_out = np.zeros(_B, np.float32)
    for c in range(_NC):
        o = res_b[c]["out"][0]
        for s in range(_SPC):
            b = c * _SPC + s
            if meta[c][s]["overflow"]:
                cls_out[b], bbox_out[b], ldm_out[b] = _np_sample(
                    cls_h[b], breg_h[b], lreg_h[b], anc_full, ann_h[b])
            else:
                cls_out[b] = o[0 + s]
                bbox_out[b] = o[2 + s]
                ldm_out[b] = o[4 + s]
    return cls_out, bbox_out, ldm_out


def _np_sample(cls, breg, lreg, anchor, ann):
    """Exact numpy fallback for a single sample (npos > K overflow)."""
    valid = ann[:, 0] > 0
    boxes = ann[:, :4]
    ldm_ann = ann[:, 4:]
    has_gt = bool(valid.any())
    area = (boxes[:, 2] - boxes[:, 0]) * (boxes[:, 3] - boxes[:, 1])
    iw = np.minimum(anchor[:, 2][:, None], boxes[None, :, 2]) - \
        np.maximum(anchor[:, 0][:, None], boxes[None, :, 0])
    ih = np.minimum(anchor[:, 3][:, None], boxes[None, :, 3]) - \
        np.maximum(anchor[:, 1][:, None], boxes[None, :, 1])
    iw = np.clip(iw, 0.0, None)
    ih = np.clip(ih, 0.0, None)
    aw = anchor[:, 2] - anchor[:, 0]
    ah = anchor[:, 3] - anchor[:, 1]
    ua = np.clip((aw * ah)[:, None] + area[None, :] - iw * ih, 1e-8, None)
    iou = np.where(valid[None, :], iw * ih / ua, -1.0)
    iou_max = iou.max(axis=1)
    iou_arg = iou.argmax(axis=1)
    negm = iou_max < 0.4
    posm = iou_max >= 0.7
    npos = int(posm.sum())
    keep = min(int(negm.sum()), 3 * npos)
    neg_losses = np.where(negm, -cls[:, 1], -np.inf)
    srt = np.sort(neg_losses)[::-1]
    neg_mean = srt[:keep].sum(dtype=np.float32) / max(keep, 1)
    pos_mean = np.where(posm, -cls[:, 0], 0.0).sum(dtype=np.float32) / \
        max(npos, 1)
    cl = (pos_mean + neg_mean) if (has_gt and npos > 0) else 0.0
    gb = boxes[iou_arg]
    gw = gb[:, 2] - gb[:, 0]
    gh = gb[:, 3] - gb[:, 1]
    gcx = gb[:, 0] + 0.5 * gw
    gcy = gb[:, 1] + 0.5 * gh
    acx = anchor[:, 0] + 0.5 * aw
    acy = anchor[:, 1] + 0.5 * ah
    with np.errstate(all='ignore'):
        btgt = np.stack([(gcx - acx) / (aw + 1e-14) * 10,
                         (gcy - acy) / (ah + 1e-14) * 10,
                         np.log(gw / aw) * 5, np.log(gh / ah) * 5], axis=1)
    d = np.abs(btgt - breg)
    sl1 = np.where(d < 1.0, 0.5 * d * d, d - 0.5)
    bl = np.where(posm[:, None], sl1, 0.0).sum(dtype=np.float32) / \
        (max(npos, 1) * 4) if (has_gt and npos > 0) else 0.0
    gl = ldm_ann[iou_arg]
    lpos = posm & (gl.sum(axis=1) > 0)
    nl = int(lpos.sum())
    even = (np.arange(196) % 2) == 0
    ctr = np.where(even[None, :], acx[:, None], acy[:, None])
    den = np.where(even[None, :], aw[:, None], ah[:, None]) + 1e-14
    s = np.concatenate([np.ones(68, np.float32), 3 * np.ones(128, np.float32)])
    dd = np.abs((gl - ctr) / den / 0.1 * s - lreg * s)
    wing = np.where(dd < _OMEGA, _OMEGA * np.log1p(dd / _EPS), dd - _WING_C)
    ll = np.where(lpos[:, None], wing, 0.0).sum(dtype=np.float32) / \
        (max(nl, 1) * 196) if (has_gt and nl > 0) else 0.0
    return np.float32(cl), np.float32(bl), np.float32(ll)
